# revision 2
# baseline (speedup 1.0000x reference)
"""Bidirectional LSTM kernel for Trainium2 (Bass/Tile), B=64 S=256 I=H=512.

8-core data-parallel: core c runs direction c//4 (0=fwd, 1=bwd) on batch
quarter c%4 (B_local=16), gates^T layout (gate rows on partitions, batch on
free dim).

Per-step structure (the serial recurrence dominates; ~64 LDW+MM pairs issue
at the ~25ns NX floor regardless of dtype — fp8 stationary measured no
faster, so everything stays bf16):
- gate PSUM banks: [f,i] fused in one bank (one sigmoid ACT op covers both),
  [g], [o] in their own banks; all double-buffered so the 3 identity-matmul
  preloads of step t+1's x-contribution (from the precomputed ring) run on
  the PE during step t's activation tail.
- recurrent MM order: f-gate k01 pairs first, deferring the first k23 reads
  ~8 MMs so the h23 half-multiply of the previous step has slack; gate order
  [f, i, g, o] so g's tanh (longest chain) starts right when ACT frees up
  and o (shortest path) lands last.
- tail: SIG(fi) -> t2=f*c | TANH(g) -> t1=i*g -> c=t1+t2 -> SIG(o) ->
  TANH(c) -> h halves (k01 first so next step's first MMs start earlier).
- ring sweeps precompute x@Wx+bias in 256-col MMs; evictions all on DVE
  (ACT is the tail-critical engine), pinned into step tails.
- HAM warmup burst + small junk-MM filler keeps the PE clock at 2.4 GHz.
"""

import numpy as np
import ml_dtypes

P = 128
B_FULL = 64     # full batch
NB = 4          # batch shards per direction
BL = B_FULL // NB  # local batch
HD = 512        # hidden dim
ID = 512        # input dim
KH = HD // P    # 4 k-chunks over h
KI = ID // P    # 4 k-chunks over x
M4 = 4 * HD // P  # 16 m-chunks over the 4*H gate dim; order [f, i, g, o]
S_FULL = 256
SWEEP_FULL = 16

_NC_CACHE = {}


def build(S=S_FULL, SWEEP=SWEEP_FULL, B=BL):
    """Build and bacc-compile the single-core LSTM program (local batch B)."""
    import concourse.bacc as bacc
    import concourse.mybir as mybir
    import concourse.tile as tile
    from concourse.tile import add_dep_helper
    from contextlib import ExitStack

    AF = mybir.ActivationFunctionType
    bf16 = mybir.dt.bfloat16
    f32 = mybir.dt.float32

    assert S % SWEEP == 0
    n_sweeps = S // SWEEP
    COLS = SWEEP * B              # columns per sweep window
    NCH = max(1, COLS // 512)     # 512-col chunks per window
    NCOL = COLS // NCH            # columns per chunk (<= 512)
    TPC = NCOL // B               # timesteps covered per chunk
    n_groups = NCH * M4           # (n, m) GEMM groups per window
    assert n_groups % SWEEP == 0 or SWEEP % n_groups == 0
    gps = max(1, n_groups // SWEEP)  # groups emitted per step

    nc = bacc.Bacc("TRN2", target_bir_lowering=False, debug=False, num_devices=8)

    xT = nc.dram_tensor("xT", (P, KI, S * B), bf16, kind="ExternalInput")
    wx = nc.dram_tensor("wx", (P, KI, M4, P), bf16, kind="ExternalInput")
    wh = nc.dram_tensor("wh", (P, KH, M4, P), bf16, kind="ExternalInput")
    bias = nc.dram_tensor("bias", (P, M4), f32, kind="ExternalInput")
    ident = nc.dram_tensor("ident", (P, P), bf16, kind="ExternalInput")
    hsT = nc.dram_tensor("hsT", (S, KH, P, B), bf16, kind="ExternalOutput")

    MH = M4 // 2  # 8: f+i m-chunks (fused sigmoid bank)

    with tile.TileContext(nc) as tc, ExitStack() as ctx:
        constp = ctx.enter_context(tc.tile_pool(name="const", bufs=1))
        xinp = ctx.enter_context(tc.tile_pool(name="xin", bufs=3))
        ringp = ctx.enter_context(tc.tile_pool(name="ring", bufs=3))
        statep = ctx.enter_context(tc.tile_pool(name="state", bufs=4))
        ewp = ctx.enter_context(tc.tile_pool(name="ew", bufs=4))
        psfi = ctx.enter_context(tc.tile_pool(name="psum_fi", bufs=2, space="PSUM"))
        psg = ctx.enter_context(tc.tile_pool(name="psum_g", bufs=2, space="PSUM"))
        pso = ctx.enter_context(tc.tile_pool(name="psum_o", bufs=2, space="PSUM"))
        psx = ctx.enter_context(tc.tile_pool(name="psum_x", bufs=2, space="PSUM"))

        wx_sb = constp.tile([P, KI, M4, P], bf16)
        wh_sb = constp.tile([P, KH, M4, P], bf16)
        for k in range(KI):
            nc.sync.dma_start(out=wx_sb[:, k], in_=wx.ap()[:, k])
        for k in range(KH):
            nc.sync.dma_start(out=wh_sb[:, k], in_=wh.ap()[:, k])
        bias_sb = constp.tile([P, M4], f32)
        nc.sync.dma_start(out=bias_sb[:], in_=bias.ap())
        id_sb = constp.tile([P, P], bf16)
        nc.sync.dma_start(out=id_sb[:], in_=ident.ap())

        x_bufs = {}
        ring_bufs = {}

        def load_x(s):
            t_ = xinp.tile([P, KI, COLS], bf16, tag="xin", name=f"xin{s}")
            nc.sync.dma_start(out=t_[:], in_=xT.ap()[:, :, s * COLS:(s + 1) * COLS])
            x_bufs[s] = t_

        def new_ring(s):
            ring_bufs[s] = ringp.tile([P, SWEEP, M4, B], bf16, tag="ring", name=f"ring{s}")

        def sweep_group(s, n, m, after=None, evict_after=None):
            xb = x_bufs[s]
            rb = ring_bufs[s]
            pt = psx.tile([P, TPC, B], f32, tag="psx")
            last = None
            for k in range(KI):
                mm = nc.tensor.matmul(
                    pt[:], wx_sb[:, k, m, :], xb[:, k, n * NCOL:(n + 1) * NCOL],
                    start=(k == 0), stop=(k == KI - 1),
                )
                if k == 0 and after is not None:
                    add_dep_helper(mm.ins, after.ins, sync=False,
                                   reason="pin sweep into step tail")
                last = mm
            ev = nc.vector.tensor_scalar_add(
                out=rb[:, n * TPC:(n + 1) * TPC, m, :], in0=pt[:],
                scalar1=bias_sb[:, m:m + 1],
            )
            if evict_after is not None:
                add_dep_helper(ev.ins, evict_after.ins, sync=False,
                               reason="evict after step chain ops")
            return last

        GW = NCH * M4
        total_groups = n_sweeps * GW
        PRO = min(total_groups, M4 + 4 * gps)

        def emit_gi(gi, after=None, evict_after=None):
            gs, rem = divmod(gi, GW)
            gn, gm = divmod(rem, M4)
            if rem == 0:
                load_x(gs)
                new_ring(gs)
            return sweep_group(gs, gn, gm, after=after, evict_after=evict_after)

        for gi in range(PRO):
            emit_gi(gi)

        # HAM warmup: ~5us of contiguous junk matmuls so the PE clock-gate
        # un-throttles (needs ~3.4us sustained busy at 1.2 GHz).
        wt = psx.tile([P, TPC, B], f32, tag="psx", name="warm")
        warm_last = None
        for wi in range(24):
            wm = nc.tensor.matmul(
                wt[:], id_sb[:], wx_sb[:, 0, 0:2, :],
                start=True, stop=True)
            if warm_last is not None:
                add_dep_helper(wm.ins, warm_last.ins, sync=False,
                               reason="warmup chain")
            warm_last = wm

        def emit_preloads(t, after=None):
            """Identity-MM preloads of the x-part for step t into fresh banks."""
            s, sl = divmod(t, SWEEP)
            rb = ring_bufs[s]
            fin = (t == 0)  # step 0 has no recurrent MMs: groups end here
            gfi = psfi.tile([P, MH, B], f32, tag="gfi")
            gg = psg.tile([P, KH, B], f32, tag="gg")
            go = pso.tile([P, KH, B], f32, tag="go")
            m0 = nc.tensor.matmul(gfi[:], id_sb[:], rb[:, sl, 0:MH, :],
                                  start=True, stop=fin)
            if after is not None:
                add_dep_helper(m0.ins, after.ins, sync=False,
                               reason="preload order")
            nc.tensor.matmul(gg[:], id_sb[:], rb[:, sl, MH:MH + KH, :],
                             start=True, stop=fin)
            m2 = nc.tensor.matmul(go[:], id_sb[:], rb[:, sl, MH + KH:M4, :],
                                  start=True, stop=fin)
            return (gfi, gg, go), m2

        # recurrent MM order: f-gate k01 for m0-3 first, then their k23 (gives
        # the previous step's h23 half ~8 MM slots of slack), then i, g, o
        # with k inner.  One stop per PSUM tile group, on its last MM.
        MM_ORDER = (
            [(m, k) for m in range(4) for k in (0, 1)]
            + [(m, k) for m in range(4) for k in (2, 3)]
            + [(m, k) for m in range(4, M4) for k in range(KH)]
        )

        pre_tiles, pre_last = emit_preloads(0, after=warm_last)

        h_prev = None
        c_prev = None
        last_sweep_mm = None
        next_gi = PRO
        for t in range(S):
            gfi, gg, go = pre_tiles

            def gp_slot(m):
                if m < MH:
                    return gfi, m
                if m < MH + KH:
                    return gg, m - MH
                return go, m - MH - KH

            last_h_mm = pre_last
            if t > 0:
                for m, k in MM_ORDER:
                    gp_t, ml = gp_slot(m)
                    is_last_of_tile = (
                        (m == MH - 1 and k == KH - 1)
                        or (m == MH + KH - 1 and k == KH - 1)
                        or (m == M4 - 1 and k == KH - 1))
                    last_h_mm = nc.tensor.matmul(
                        gp_t[:, ml, :], wh_sb[:, k, m, :], h_prev[:, k, :],
                        start=False, stop=is_last_of_tile)

            sfi = ewp.tile([P, MH, B], bf16, tag="sfi")
            i_sfi = nc.scalar.activation(sfi[:], gfi[:], AF.Sigmoid)
            if t > 0:
                t2 = ewp.tile([P, KH, B], bf16, tag="t2")
                i_t2 = nc.vector.tensor_mul(out=t2[:], in0=sfi[:, 0:KH, :],
                                            in1=c_prev[:])
            tg = ewp.tile([P, KH, B], bf16, tag="tg")
            i_tg = nc.scalar.activation(tg[:], gg[:], AF.Tanh)
            add_dep_helper(i_tg.ins, i_sfi.ins, sync=False, reason="act order")
            t1 = ewp.tile([P, KH, B], bf16, tag="t1")
            i_t1 = nc.vector.tensor_mul(out=t1[:], in0=sfi[:, KH:MH, :], in1=tg[:])
            if t > 0:
                add_dep_helper(i_t1.ins, i_t2.ins, sync=False, reason="dve order")
            so = ewp.tile([P, KH, B], bf16, tag="so")
            i_so = nc.scalar.activation(so[:], go[:], AF.Sigmoid)
            add_dep_helper(i_so.ins, i_tg.ins, sync=False, reason="act order")

            c_new = statep.tile([P, KH, B], bf16, tag="c")
            if t == 0:
                i_cn = nc.vector.tensor_copy(out=c_new[:], in_=t1[:])
            else:
                i_cn = nc.vector.tensor_add(out=c_new[:], in0=t1[:], in1=t2[:])
            add_dep_helper(i_cn.ins, i_t1.ins, sync=False, reason="dve order")
            tct = ewp.tile([P, KH, B], bf16, tag="tct")
            tct_inst = nc.scalar.activation(tct[:], c_new[:], AF.Tanh)
            add_dep_helper(tct_inst.ins, i_so.ins, sync=False, reason="act order")
            h_new = statep.tile([P, KH, B], bf16, tag="hT")
            # split the final h multiply so the next step's k0/k1 matmuls can
            # start one DVE-op earlier (they only read h chunks 0-1)
            HH = KH // 2
            hmul_a = nc.vector.tensor_mul(
                out=h_new[:, 0:HH, :], in0=so[:, 0:HH, :], in1=tct[:, 0:HH, :])
            add_dep_helper(hmul_a.ins, i_cn.ins, sync=False, reason="dve order")
            hmul_inst = nc.vector.tensor_mul(
                out=h_new[:, HH:KH, :], in0=so[:, HH:KH, :], in1=tct[:, HH:KH, :])
            add_dep_helper(hmul_inst.ins, hmul_a.ins, sync=False,
                           reason="h halves order")
            nc.sync.dma_start(out=hsT.ap()[t].rearrange("k p b -> p k b"), in_=h_new[:])

            h_prev, c_prev = h_new, c_new

            # sweeps + preloads for t+1 + keep-warm filler, all queued behind
            # this step's recurrent MMs so they fill the PE during the tail.
            if next_gi < total_groups:
                for _ in range(gps):
                    if next_gi >= total_groups:
                        break
                    last_sweep_mm = emit_gi(
                        next_gi, after=last_h_mm, evict_after=hmul_inst)
                    next_gi += 1
            else:
                last_sweep_mm = None

            if t + 1 < S:
                pre_tiles, pre_last = emit_preloads(
                    t + 1, after=(last_sweep_mm or last_h_mm))

            # Keep-warm filler: junk matmuls pinned into the step tail so the
            # HAM activity window never reads mostly-idle.
            njunk = 5
            prev_pe = pre_last if t + 1 < S else (last_sweep_mm or last_h_mm)
            for _ in range(njunk):
                jm = nc.tensor.matmul(
                    wt[:], id_sb[:], wx_sb[:, 0, 0:2, :],
                    start=True, stop=True)
                add_dep_helper(jm.ins, prev_pe.ins, sync=False,
                               reason="junk in tail")
                prev_pe = jm

    nc.compile()
    return nc


def _get_nc(S, SWEEP, B=BL):
    key = (S, SWEEP, B)
    if key not in _NC_CACHE:
        _NC_CACHE[key] = build(S, SWEEP, B)
    return _NC_CACHE[key]


def prep_core_inputs(x, Wc, bc, Wi, bi, Wf, bf, Wo, bo, reverse):
    """Pack one direction's inputs into the kernel's layouts. x: (B, S, I) f32."""
    bft = ml_dtypes.bfloat16
    if reverse:
        x = x[:, ::-1, :]
    S = x.shape[1]
    B = x.shape[0]
    Wcat = np.concatenate([Wf, Wi, Wc, Wo], axis=1)      # (I+H, 4H), gate order [f,i,g,o]
    bcat = np.concatenate([bf, bi, bc, bo]).astype(np.float32)
    Wx, Wh = Wcat[:ID], Wcat[ID:]

    xT = (
        x.transpose(2, 1, 0)                  # (I, S, B)
        .reshape(KI, P, S * B)
        .transpose(1, 0, 2)                   # (P, KI, S*B)
    )
    wxp = Wx.reshape(KI, P, M4, P).transpose(1, 0, 2, 3)
    whp = Wh.reshape(KH, P, M4, P).transpose(1, 0, 2, 3)
    biasp = bcat.reshape(M4, P).T
    return {
        "xT": np.ascontiguousarray(xT).astype(bft),
        "wx": np.ascontiguousarray(wxp).astype(bft),
        "wh": np.ascontiguousarray(whp).astype(bft),
        "bias": np.ascontiguousarray(biasp),
        "ident": np.eye(P, dtype=bft),
    }


def run_lstm(x, Wi_f, bi_f, Wf_f, bf_f, Wc_f, bc_f, Wo_f, bo_f,
             Wi_b, bi_b, Wf_b, bf_b, Wc_b, bc_b, Wo_b, bo_b,
             trace=False, trace_cores=None):
    from concourse import bass_utils

    x = np.asarray(x, dtype=np.float32)
    S = x.shape[1]
    nc = _get_nc(S, SWEEP_FULL if S % SWEEP_FULL == 0 else S)
    ims = []
    for c in range(2 * NB):
        d = c // NB
        q = c % NB
        xq = x[q * BL:(q + 1) * BL]
        if d == 0:
            ims.append(prep_core_inputs(
                xq, Wc_f, bc_f, Wi_f, bi_f, Wf_f, bf_f, Wo_f, bo_f, False))
        else:
            ims.append(prep_core_inputs(
                xq, Wc_b, bc_b, Wi_b, bi_b, Wf_b, bf_b, Wo_b, bo_b, True))
    res = bass_utils.run_bass_kernel_spmd(
        nc, ims, core_ids=list(range(2 * NB)), trace=trace, trace_cores=trace_cores,
    )
    outs = []
    for c in range(2 * NB):
        hs = res.results[c]["hsT"].astype(np.float32)   # (S, KH, P, BL)
        if c // NB == 1:
            hs = hs[::-1]
        outs.append(hs.transpose(0, 3, 1, 2).reshape(S, BL, HD))  # (S, BL, H)
    fwd = np.concatenate(outs[:NB], axis=1)   # (S, B, H)
    bwd = np.concatenate(outs[NB:], axis=1)
    out = np.concatenate([fwd, bwd], axis=2).transpose(1, 0, 2)  # (B, S, 2H)
    return np.ascontiguousarray(out), res


def kernel(x, Wi_f, bi_f, Wf_f, bf_f, Wc_f, bc_f, Wo_f, bo_f,
           Wi_b, bi_b, Wf_b, bf_b, Wc_b, bc_b, Wo_b, bo_b):
    out, _ = run_lstm(x, Wi_f, bi_f, Wf_f, bf_f, Wc_f, bc_f, Wo_f, bo_f,
                      Wi_b, bi_b, Wf_b, bf_b, Wc_b, bc_b, Wo_b, bo_b)
    return out
